# revision 12
# baseline (speedup 1.0000x reference)
"""Trainium2 Bass kernel for nn_LocatorReaderConditioner (cross-attention block).

Reference computation (per batch b):
    q = query @ Wq, k = mem @ Wk, v = mem @ Wv   (split into 16 heads of 64)
    scores = q k^T / sqrt(64) + bias[None, :]
    out = softmax(scores) v   (concat heads)  @ Wo

Sharding over 8 cores: core c handles batch b = c // 2 and head-group
hg = c % 2 (8 heads, 512 feature columns of Wq/Wk/Wv, 512 rows of Wo).
Each core returns a partial output (its head-group pushed through its Wo
rows); the host sums the two partials per batch (the "all-reduce").

On-device layout: activations are kept feature-major ([feat, tokens]) so
chained matmuls need no transposes. The only transposes are the XBAR
DMA-transposed loads of query/memory. Scores are computed transposed
([ktok, q]) so the softmax exp is a single fused ACT op per tile
(exp(scale*s + bias_k) with per-partition bias); no max-subtraction is
needed (score magnitudes are ~10 for this model). The softmax denominator
comes for free from a ones-column appended to each head's V block (the AV
matmul yields [attT; denom] in one accumulation). Normalization happens
off-psum (reciprocal + partition-broadcast multiply), hidden under the
next head-pair's compute; the output projection is emitted in two phases
(heads 0-5 first) so the tensor engine never stalls on the last pair's
normalize chain.
"""

from contextlib import ExitStack

import numpy as np
import ml_dtypes

import bass_rust
import concourse.bass as bass
import concourse.mybir as mybir
import concourse.tile as tile
from concourse import bacc
from concourse.bass import ds, ts
from concourse.bass_utils import run_bass_kernel_spmd

BF16 = ml_dtypes.bfloat16
F32 = np.float32

B, Q, KT, D = 4, 512, 4096, 1024
H_PER_CORE = 8          # heads per core
DH = 64                 # head dim
DG = 512                # feature columns per core (H_PER_CORE * DH)
SCALE = DH ** -0.5
N_CORES = 8
KTILES = KT // 128      # 32
KCHUNK = 8              # din tiles (D / 128)

_CACHE: dict = {}


def _build_nc():
    nc = bacc.Bacc("TRN2", target_bir_lowering=False, debug=False)
    dt = mybir.dt

    mem = nc.dram_tensor("mem", [KT, D], dt.bfloat16, kind="ExternalInput")
    qry = nc.dram_tensor("qry", [Q, D], dt.bfloat16, kind="ExternalInput")
    wq = nc.dram_tensor("wq", [128, KCHUNK, DG], dt.bfloat16, kind="ExternalInput")
    wk = nc.dram_tensor("wk", [128, KCHUNK, DG], dt.bfloat16, kind="ExternalInput")
    wv = nc.dram_tensor("wv", [128, KCHUNK, DG], dt.bfloat16, kind="ExternalInput")
    wo = nc.dram_tensor("wo", [64, H_PER_CORE, D], dt.bfloat16, kind="ExternalInput")
    biasT = nc.dram_tensor("biasT", [128, KTILES], dt.float32, kind="ExternalInput")
    out = nc.dram_tensor("out", [Q, D], dt.float32, kind="ExternalOutput")

    with tile.TileContext(nc) as tc, ExitStack() as ctx:
        const = ctx.enter_context(tc.tile_pool(name="const", bufs=1))

        # persistent SBUF tensors
        memT = const.tile([128, KCHUNK, KT], dt.bfloat16)      # mem^T  (din, ktok)
        kT = const.tile([128, 4, KT], dt.bfloat16)             # K^T    (dout, ktok)
        v_aug = const.tile([128, KTILES, 8 * 65], dt.bfloat16)  # V nat + ones cols
        qT = const.tile([128, 4, Q], dt.bfloat16)              # Q^T    (dout, q)
        wk_s = const.tile([128, KCHUNK, DG], dt.bfloat16)
        wo_s = const.tile([64, H_PER_CORE, D], dt.bfloat16)
        bias_s = const.tile([128, KTILES], dt.float32)
        # per-head attention outputs: SEPARATE tiles so the output projection
        # only depends on the heads it reads (Tile deps are tile-granular)
        attTs = [
            const.tile([64, Q], dt.bfloat16, name=f"attn_{h}")
            for h in range(H_PER_CORE)
        ]

        early_copies = [
            nc.gpsimd.dma_start(wk_s[:, 0:4, :], wk.ap()[:, 0:4, :]),
            nc.scalar.dma_start(wk_s[:, 4:8, :], wk.ap()[:, 4:8, :]),
        ]

        # ones column LAST in each 65-wide head block: the AV matmul then
        # yields [attT rows 0-63; denominator row 64] in one accumulation
        v_aug_blocks = v_aug.rearrange("p k (h c) -> p k h c", c=65)
        nc.vector.memset(v_aug_blocks[:, :, :, 64:65], 1.0)

        ps_stack = ExitStack()
        # proj_ps lives on the OUTER scope: the first two Wo psum groups keep
        # accumulating in it after the attention pools (ps_stack) close.
        proj_ps = ctx.enter_context(
            tc.tile_pool(name="proj_ps", bufs=2, space="PSUM")
        )

        with tc.tile_pool(name="early", bufs=1) as early:
            queryT = early.tile([128, KCHUNK, Q], dt.bfloat16)
            wq_s = early.tile([128, KCHUNK, DG], dt.bfloat16)
            wv_s = early.tile([128, KCHUNK, DG], dt.bfloat16)
            early_copies.append(nc.gpsimd.dma_start(wq_s[:, 0:4, :], wq.ap()[:, 0:4, :]))
            early_copies.append(nc.scalar.dma_start(wq_s[:, 4:8, :], wq.ap()[:, 4:8, :]))
            early_copies.append(nc.gpsimd.dma_start(wv_s[:, 0:4, :], wv.ap()[:, 0:4, :]))
            early_copies.append(nc.scalar.dma_start(wv_s[:, 4:8, :], wv.ap()[:, 4:8, :]))

            # XBAR transposes. Tile serializes every copy<->transpose
            # xbar-mode transition, so order the DMAs to pay only two
            # transitions: [wk wq wv copies] -> [all transposes] -> [wo bias
            # copies]. Without explicit deps the scheduler interleaves them
            # into a long serialized ping-pong chain.
            xbars = [nc.sync.dma_start(out=queryT, in_=qry.ap(), transpose=True)]
            for c in range(8):
                xbars.append(
                    nc.sync.dma_start(
                        out=memT[:, :, ts(c, KT // 8)],
                        in_=mem.ap()[ds(c * (KT // 8), KT // 8), :],
                        transpose=True,
                    )
                )
            for x in xbars:
                for cd in early_copies:
                    bass_rust.add_dep_helper(
                        x.ins, cd.ins, sync=True,
                        reason="xbar transposes after early copy DMAs",
                    )
            late_copies = [
                nc.gpsimd.dma_start(wo_s, wo.ap()),
                nc.gpsimd.dma_start(bias_s, biasT.ap()),
            ]
            for cd in late_copies:
                bass_rust.add_dep_helper(
                    cd.ins, xbars[-1].ins, sync=True,
                    reason="late copy DMAs after xbar transposes",
                )

            # ---- Q projection: qT[dout, q] ----
            for mt in range(4):
                ps = proj_ps.tile([128, Q], dt.float32, tag="proj")
                for kt in range(KCHUNK):
                    nc.tensor.matmul(
                        ps, wq_s[:, kt, ts(mt, 128)], queryT[:, kt, :],
                        start=(kt == 0), stop=(kt == KCHUNK - 1),
                    )
                nc.vector.tensor_copy(qT[:, mt, :], ps)

            # ---- K proj (dout tile 0) + V proj, pipelined by ktok chunk ----
            for c in range(8):
                ps = proj_ps.tile([128, 512], dt.float32, tag="proj")
                for kt in range(KCHUNK):
                    nc.tensor.matmul(
                        ps, wk_s[:, kt, ts(0, 128)], memT[:, kt, ts(c, 512)],
                        start=(kt == 0), stop=(kt == KCHUNK - 1),
                    )
                nc.vector.tensor_copy(kT[:, 0, ts(c, 512)], ps)
                for ktile in range(4 * c, 4 * c + 4):   # V natural rows
                    ps = proj_ps.tile([128, DG], dt.float32, tag="proj")
                    for kt in range(KCHUNK):
                        nc.tensor.matmul(
                            ps, memT[:, kt, ts(ktile, 128)], wv_s[:, kt, :],
                            start=(kt == 0), stop=(kt == KCHUNK - 1),
                        )
                    nc.vector.tensor_copy(
                        v_aug_blocks[:, ktile, :, 0:64],
                        ps.rearrange("p (h c) -> p h c", c=64),
                    )

        # ---- attention, one head-pair at a time ----
        scores_ps = ps_stack.enter_context(
            tc.tile_pool(name="scores_ps", bufs=2, space="PSUM"))
        av_ps = ps_stack.enter_context(
            tc.tile_pool(name="av_ps", bufs=1, space="PSUM"))
        p_pool = ps_stack.enter_context(tc.tile_pool(name="p_pool", bufs=3))
        norm_pool = ps_stack.enter_context(tc.tile_pool(name="norm_pool", bufs=1))

        LAG = 2
        for pair in range(4):
            h0, h1 = 2 * pair, 2 * pair + 1
            av = [
                av_ps.tile([65, Q], dt.float32, tag="av0", name=f"av0_{pair}"),
                av_ps.tile([65, Q], dt.float32, tag="av1", name=f"av1_{pair}"),
            ]
            pending = []

            def flush_av(k, p_sb):
                nc.tensor.matmul(
                    av[0], v_aug_blocks[:, k, h0, :], p_sb[:, 0:Q],
                    start=(k == 0), stop=(k == KTILES - 1),
                )
                nc.tensor.matmul(
                    av[1], v_aug_blocks[:, k, h1, :], p_sb[:, Q:2 * Q],
                    start=(k == 0), stop=(k == KTILES - 1),
                )

            for k in range(KTILES):
                sc = scores_ps.tile([128, 2 * Q], dt.float32, tag="sc")
                # row-packed pair: even head on array rows 0-63, odd on 64-127
                nc.tensor.matmul(
                    sc[:, 0:Q], kT[0:64, pair, ts(k, 128)], qT[0:64, pair, :],
                    start=True, stop=True,
                )
                nc.tensor.matmul(
                    sc[:, Q:2 * Q], kT[64:128, pair, ts(k, 128)], qT[64:128, pair, :],
                    start=True, stop=True,
                )
                p_sb = p_pool.tile([128, 2 * Q], dt.bfloat16, tag="p")
                nc.scalar.activation(
                    p_sb, sc, mybir.ActivationFunctionType.Exp,
                    bias=bias_s[:, k:k + 1], scale=SCALE,
                )
                pending.append((k, p_sb))
                # interleave next K-proj dout tile to keep PE fed during exp
                if pair < 3 and k % 4 == 3:
                    nk = k // 4
                    mt = pair + 1
                    ps = proj_ps.tile([128, 512], dt.float32, tag="proj")
                    for kt in range(KCHUNK):
                        nc.tensor.matmul(
                            ps, wk_s[:, kt, ts(mt, 128)], memT[:, kt, ts(nk, 512)],
                            start=(kt == 0), stop=(kt == KCHUNK - 1),
                        )
                    nc.vector.tensor_copy(kT[:, mt, ts(nk, 512)], ps)
                if len(pending) > LAG:
                    flush_av(*pending.pop(0))
            while pending:
                flush_av(*pending.pop(0))

            # Evacuate av psum fast (2 cheap DVE copies) so the next pair's
            # AV matmuls aren't blocked, then normalize off-psum, hidden
            # under subsequent compute. partition_broadcast on HW broadcasts
            # PHYSICAL partition 0 (ignores AP offsets), hence the tiny
            # SBUF->SBUF DMA hop of the denominator row to a partition-0 tile.
            for par, h in ((0, h0), (1, h1)):
                den64 = norm_pool.tile([65, Q], dt.float32, tag="den64", bufs=2,
                                       name=f"den64_{h}")
                nc.vector.tensor_copy(attTs[h][:, :], av[par][0:64, :])
                nc.vector.tensor_copy(den64[64:65, :], av[par][64:65, :])
                rec0 = norm_pool.tile([1, Q], dt.float32, tag="rec0", bufs=2,
                                      name=f"rec0_{h}")
                nc.gpsimd.dma_start(rec0, den64[64:65, :])
                recv = norm_pool.tile([1, Q], dt.float32, tag="recv", bufs=2,
                                      name=f"recv_{h}")
                nc.vector.reciprocal(recv, rec0)
                rec_bc = norm_pool.tile([64, Q], dt.float32, tag="rec_bc", bufs=2,
                                        name=f"rec_bc_{h}")
                nc.gpsimd.partition_broadcast(rec_bc, recv)
                nc.vector.tensor_mul(attTs[h][:, :], attTs[h][:, :], rec_bc)

        # ---- output projection: K=64 per head, accumulated over heads.
        # Two-phase emission: heads 0-5 for all 8 psum groups first, so the
        # PE has work while the last pair's normalize chain completes. The
        # first two groups draw psum from proj_ps (its banks have been idle
        # since pair 2's K-projection finished) so the PE can start on them
        # instantly instead of waiting for the attention pools' bank release.
        groups = []

        def wo_phase1(ps_pool, qt_i, dt_i, tag=None):
            ps = ps_pool.tile([128, 512], mybir.dt.float32,
                              tag=tag or f"wo_{qt_i}_{dt_i}",
                              name=f"wo_ps_{qt_i}_{dt_i}")
            for h in range(6):
                nc.tensor.matmul(
                    ps, attTs[h][:, ts(qt_i, 128)],
                    wo_s[:, h, ts(dt_i, 512)],
                    start=(h == 0), stop=False,
                )
            groups.append((qt_i, dt_i, ps))

        # reuse the two "proj" slots (idle since pair 2's K-projection)
        wo_phase1(proj_ps, 0, 0, tag="proj")
        wo_phase1(proj_ps, 0, 1, tag="proj")

        # free all attention-phase psum pools so the rest of Wo has banks
        ps_stack.close()

        with tc.tile_pool(name="wo_ps", bufs=1, space="PSUM") as wo_ps, \
                tc.tile_pool(name="out_pool", bufs=2) as out_pool:
            for qt_i in range(4):
                for dt_i in range(2):
                    if (qt_i, dt_i) in ((0, 0), (0, 1)):
                        continue
                    wo_phase1(wo_ps, qt_i, dt_i)
            for qt_i, dt_i, ps in groups:
                for h in (6, 7):
                    nc.tensor.matmul(
                        ps, attTs[h][:, ts(qt_i, 128)],
                        wo_s[:, h, ts(dt_i, 512)],
                        start=False, stop=(h == 7),
                    )
                o_sb = out_pool.tile([128, 512], mybir.dt.float32, tag="o",
                                     name=f"o_sb_{qt_i}_{dt_i}")
                nc.vector.tensor_copy(o_sb, ps)
                nc.scalar.dma_start(
                    out.ap()[ds(qt_i * 128, 128), ds(dt_i * 512, 512)], o_sb
                )

    nc.compile()
    return nc


def _shard_inputs(query, memory, bias, Wq, Wk, Wv, Wo):
    """Host-side sharding + layout packing (per-core input dicts)."""
    in_maps = []
    packed = {}
    for hg in range(2):
        cols = slice(hg * DG, (hg + 1) * DG)
        packed[hg] = {
            "wq": np.ascontiguousarray(
                Wq[:, cols].reshape(KCHUNK, 128, DG).transpose(1, 0, 2)
            ).astype(BF16),
            "wk": np.ascontiguousarray(
                Wk[:, cols].reshape(KCHUNK, 128, DG).transpose(1, 0, 2)
            ).astype(BF16),
            "wv": np.ascontiguousarray(
                Wv[:, cols].reshape(KCHUNK, 128, DG).transpose(1, 0, 2)
            ).astype(BF16),
            "wo": np.ascontiguousarray(
                Wo[cols, :].reshape(H_PER_CORE, 64, D).transpose(1, 0, 2)
            ).astype(BF16),
        }
    for core in range(N_CORES):
        b, hg = core // 2, core % 2
        in_maps.append(
            {
                "mem": memory[b].astype(BF16),
                "qry": query[b].astype(BF16),
                "biasT": np.ascontiguousarray(bias[b].reshape(KTILES, 128).T).astype(F32),
                **packed[hg],
            }
        )
    return in_maps


def _get_nc():
    if "nc" not in _CACHE:
        _CACHE["nc"] = _build_nc()
    return _CACHE["nc"]


def run_sharded(inputs: dict, **run_kwargs):
    """Shard, run on 8 cores, gather. Returns (output, BassKernelResults)."""
    nc = _get_nc()
    in_maps = _shard_inputs(
        inputs["query"], inputs["memory"], inputs["bias"],
        inputs["Wq"], inputs["Wk"], inputs["Wv"], inputs["Wo"],
    )
    res = run_bass_kernel_spmd(nc, in_maps, core_ids=list(range(N_CORES)), **run_kwargs)
    out = np.empty((B, Q, D), dtype=F32)
    for b in range(B):
        out[b] = res.results[2 * b]["out"] + res.results[2 * b + 1]["out"]
    return out, res


def kernel(**inputs) -> np.ndarray:
    inputs = {k: np.asarray(v) for k, v in inputs.items()}
    out, _ = run_sharded(inputs)
    return out
